# revision 1
# baseline (speedup 1.0000x reference)
"""TRN2 Bass kernel v3 for CrossOpLayerUTPM — batch-sharded, (i,d)-expanded.

out[b,(i,j)] = x[b,i] x[b,j] s[i,j].  Diagonals d=j-i in 8 chunks of 32.
Column layout m = OFFC[c] + i*32 + dd  (d = 1+32c+dd).

Per (chunk, batch-tile) two plain-2D bf16 DVE ops (all partition-base 0,
all contiguous free ranges — the only fast DVE paths on cayman):
    t1 = xE[:, 32*d0 : 32*(d0+W)] * xR[:, 0:32W]     # x[b,i+d] * x[b,i]
    o  = t1 * S[:, chunk]                             # * s[i,i+d]
xE[b, m*32+t] = x[b, m+t] (host-built sliding window), xR[b, i*32+dd] =
x[b, i] (host-built repeat), S broadcast rows (host).  bf16 out; host
drops garbage columns (j>255) and reorders pairs.
"""
import numpy as np
import ml_dtypes
from contextlib import ExitStack

import jax
from jax.sharding import Mesh, PartitionSpec
from jax.experimental.shard_map import shard_map

import concourse.bass as bass
import concourse.bacc as bacc
import concourse.tile as tile
from concourse import mybir
from concourse.bass2jax import (
    _bass_exec_p,
    install_neuronx_cc_hook,
    partition_id_tensor,
)

F32 = mybir.dt.float32
BF16 = mybir.dt.bfloat16
BF16NP = ml_dtypes.bfloat16

B, NCOL = 4096, 256
NCORES = 8
BPC = B // NCORES        # 512
NT = BPC // 128          # 4
NCH = 8
D0 = [1 + 32 * c for c in range(NCH)]
WC = [NCOL - d0 for d0 in D0]
OFFC = np.concatenate([[0], np.cumsum([32 * w for w in WC])]).astype(np.int64)
TOTF = int(OFFC[-1])     # 36608
XE_LEN = NCOL * 32       # 8192


def _build_nc(reps=1):
    nc = bacc.Bacc("TRN2", target_bir_lowering=False, debug=False)
    xe_in = nc.dram_tensor("xe", [BPC, XE_LEN], BF16, kind="ExternalInput")
    xr_in = nc.dram_tensor("xr", [BPC, XE_LEN], BF16, kind="ExternalInput")
    s_in = nc.dram_tensor("sb", [128, TOTF], BF16, kind="ExternalInput")
    out_t = nc.dram_tensor("out", [BPC, TOTF], BF16, kind="ExternalOutput")

    with tile.TileContext(nc) as tc, ExitStack() as ctx:
        cpool = ctx.enter_context(tc.tile_pool(name="const", bufs=1))
        xpool = ctx.enter_context(tc.tile_pool(name="xtiles", bufs=2))
        work = ctx.enter_context(tc.tile_pool(name="work", bufs=2))

        s_all = cpool.tile([128, TOTF], BF16, name="s_all")
        nc.sync.dma_start(out=s_all[:, :], in_=s_in[:, :])

        for r in range(reps):
          for t in range(NT):
            xe = xpool.tile([128, XE_LEN], BF16, tag="xe", name=f"xe{r}_{t}",
                            bufs=2)
            nc.sync.dma_start(out=xe[:, :], in_=xe_in[t * 128:(t + 1) * 128, :])
            xr = xpool.tile([128, XE_LEN], BF16, tag="xr", name=f"xr{r}_{t}",
                            bufs=2)
            nc.sync.dma_start(out=xr[:, :], in_=xr_in[t * 128:(t + 1) * 128, :])
            for c in range(NCH):
                d0, w = D0[c], WC[c]
                for h in range(2):
                    wlo = (w // 2) * h
                    whi = w if h else (w // 2)
                    fsz = 32 * (whi - wlo)
                    off = int(OFFC[c]) + 32 * wlo
                    exo = 32 * (d0 + wlo)
                    t1 = work.tile([128, 4096], BF16, tag="t1",
                                   name=f"t{r}_{c}_{t}_{h}", bufs=3)
                    nc.vector.tensor_mul(
                        t1[:, 0:fsz], xe[:, exo:exo + fsz],
                        xr[:, 32 * wlo:32 * wlo + fsz])
                    o = work.tile([128, 4096], BF16, tag="o",
                                  name=f"o{r}_{c}_{t}_{h}", bufs=3)
                    nc.vector.tensor_mul(o[:, 0:fsz], t1[:, 0:fsz],
                                         s_all[:, off:off + fsz])
                    nc.sync.dma_start(
                        out=out_t[t * 128:(t + 1) * 128, off:off + fsz],
                        in_=o[:, 0:fsz])

    nc.compile()
    return nc


class _Runner:
    def __init__(self, nc, n_cores=NCORES):
        install_neuronx_cc_hook()
        self.nc = nc
        self.n_cores = n_cores
        partition_name = (
            nc.partition_id_tensor.name if nc.partition_id_tensor else None
        )
        in_names, out_names, out_avals, zero_outs = [], [], [], []
        for alloc in nc.m.functions[0].allocations:
            if not isinstance(alloc, mybir.MemoryLocationSet):
                continue
            name = alloc.memorylocations[0].name
            if alloc.kind == "ExternalInput":
                if name != partition_name:
                    in_names.append(name)
            elif alloc.kind == "ExternalOutput":
                shape = tuple(alloc.tensor_shape)
                dtype = mybir.dt.np(alloc.dtype)
                out_avals.append(jax.core.ShapedArray(shape, dtype))
                zero_outs.append(np.zeros(shape, dtype))
                out_names.append(name)
        self.n_params = len(in_names)
        self.param_names = list(in_names)
        self.out_names = out_names
        self.out_avals = out_avals
        self.zero_outs = zero_outs
        all_in = in_names + out_names
        if partition_name is not None:
            all_in.append(partition_name)

        def _body(*args):
            operands = list(args)
            if partition_name is not None:
                operands.append(partition_id_tensor())
            return tuple(_bass_exec_p.bind(
                *operands,
                out_avals=tuple(out_avals),
                in_names=tuple(all_in),
                out_names=tuple(out_names),
                lowering_input_output_aliases=(),
                sim_require_finite=False,
                sim_require_nnan=False,
                nc=nc,
            ))

        devices = jax.devices()[:n_cores]
        mesh = Mesh(np.asarray(devices), ("core",))
        n_outs = len(out_names)
        in_specs = (PartitionSpec("core"),) * (self.n_params + n_outs)
        out_specs = (PartitionSpec("core"),) * n_outs
        self.fn = jax.jit(
            shard_map(_body, mesh=mesh, in_specs=in_specs,
                      out_specs=out_specs, check_rep=False),
            keep_unused=True,
        )

    def run_concat(self, concat_in):
        concat_zeros = [
            np.zeros((self.n_cores * z.shape[0], *z.shape[1:]), z.dtype)
            for z in self.zero_outs
        ]
        outs = self.fn(*concat_in, *concat_zeros)
        return [np.asarray(o) for o in outs]


_CACHE = {}


def _get_runner(reps=1):
    if reps not in _CACHE:
        _CACHE[reps] = _Runner(_build_nc(reps))
    return _CACHE[reps]


def _host_prep(x, latent_emb):
    x = np.asarray(x, np.float32)
    L = np.asarray(latent_emb, np.float32)
    s = (L @ L.T).astype(np.float32)

    # S_flat[(c, i, dd)] = s[i, i + 1 + 32c + dd] (0 where j > 255)
    s_flat = np.zeros(TOTF, np.float32)
    for c in range(NCH):
        d0, w = D0[c], WC[c]
        ii, dd = np.meshgrid(np.arange(w), np.arange(32), indexing="ij")
        j = ii + d0 + dd
        blk = np.zeros((w, 32), np.float32)
        valid = j <= NCOL - 1
        blk[valid] = s[ii[valid], j[valid]]
        s_flat[OFFC[c]:OFFC[c + 1]] = blk.reshape(-1)
    s_bcast = np.broadcast_to(s_flat.astype(BF16NP), (128, TOTF)).copy()

    # per-core xE (sliding windows) and xR (32x repeat)
    xb = x.astype(BF16NP)
    xpad = np.zeros((B, NCOL + 32), BF16NP)
    xpad[:, :NCOL] = xb
    win = np.lib.stride_tricks.sliding_window_view(
        xpad, 32, axis=1)[:, :NCOL, :]                    # [B, 256, 32]
    xE = win.reshape(B, XE_LEN)
    xR = np.repeat(xb, 32, axis=1)                        # [B, 8192]

    xe_cores = [np.ascontiguousarray(xE[c * BPC:(c + 1) * BPC])
                for c in range(NCORES)]
    xr_cores = [np.ascontiguousarray(xR[c * BPC:(c + 1) * BPC])
                for c in range(NCORES)]
    return xe_cores, xr_cores, s_bcast


_IDX = None


def _pair_index():
    global _IDX
    if _IDX is None:
        iu, ju = np.triu_indices(NCOL, k=1)
        d = ju - iu
        c = (d - 1) // 32
        dd = d - 1 - 32 * c
        _IDX = (OFFC[c] + iu * 32 + dd).astype(np.int64)
    return _IDX


def kernel(x, latent_emb):
    xe_cores, xr_cores, s_bcast = _host_prep(x, latent_emb)
    runner = _get_runner()
    concat_in = []
    for name in runner.param_names:
        if name == "xe":
            concat_in.append(np.concatenate(xe_cores, axis=0))
        elif name == "xr":
            concat_in.append(np.concatenate(xr_cores, axis=0))
        elif name == "sb":
            concat_in.append(np.concatenate([s_bcast] * NCORES, axis=0))
        else:
            raise KeyError(name)
    outs = runner.run_concat(concat_in)
    dev = outs[runner.out_names.index("out")]     # [4096, TOTF] bf16
    return dev[:, _pair_index()].astype(np.float32)



# revision 2
# speedup vs baseline: 1.3245x; 1.3245x over previous
"""TRN2 Bass kernel v4 for CrossOpLayerUTPM — wrapped-diagonal layout.

out[b, (d, i)] = x[b, i] * x[b, (i+d)%256] * s[i, (i+d)%256]
for d = 1..128 (d=1..127 full wrap covers every unordered pair once;
d=128 valid for i<128 only, rest zeroed via s). Zero padding waste.

Per 128-row batch tile, per parity (odd/even d) and col-chunk:
  op1: t1 = window-AP(xsrc) * bcast-AP(xsrc)      # x[b,i+d] * x[b,i]
  op2: o  = t1 * s_b                              # * s
Window reads come straight from a [128, 768] x-source tile (cols 0:384
even-d source x[m%256], cols 384:768 odd-d source x[(m+1)%256]) so every
run start is 4B-aligned and no expanded operands are ever materialized.
s_b [128, 32768] is built on-device by partition-broadcast DMA.
"""
import numpy as np
import ml_dtypes
from contextlib import ExitStack

import jax
from jax.sharding import Mesh, PartitionSpec
from jax.experimental.shard_map import shard_map

import concourse.bass as bass
import concourse.bacc as bacc
import concourse.tile as tile
from concourse import mybir
from concourse.ap import AP
from concourse.bass2jax import (
    _bass_exec_p,
    install_neuronx_cc_hook,
    partition_id_tensor,
)

F32 = mybir.dt.float32
BF16 = mybir.dt.bfloat16
BF16NP = ml_dtypes.bfloat16

B, NCOL = 4096, 256
NCORES = 8
BPC = B // NCORES        # 512
NT = BPC // 128          # 4
ND = 64                  # d-blocks per parity
NI = NCOL                # 256
HALF = ND * NI           # 16384
TOTF = 2 * HALF          # 32768
SRC = 384                # per-parity source columns
NCHUNK = 2               # col chunks per parity half
CF = HALF // NCHUNK      # 8192


def _wap(t, offset, dims):
    base = t[:, :]
    part = base.ap[0]
    return AP(tensor=base.tensor, offset=base.offset + offset,
              ap=[list(part)] + [list(d) for d in dims])


def _build_nc(reps=1, n_off=0):
    """n_off (0..32): tail d-blocks of each parity whose op2 runs on GPSIMD,
    fully decoupled (dedicated t1g/og tiles so GPSIMD lag never stalls DVE).
    """
    GK = n_off                  # gpsimd d-blocks per parity
    DK = ND // NCHUNK - GK      # DVE d-blocks in the last chunk
    nc = bacc.Bacc("TRN2", target_bir_lowering=False, debug=False)
    xs_in = nc.dram_tensor("xs", [BPC, 2 * SRC], BF16, kind="ExternalInput")
    s_in = nc.dram_tensor("srow", [1, TOTF], BF16, kind="ExternalInput")
    out_t = nc.dram_tensor("out", [BPC, TOTF], BF16, kind="ExternalOutput")

    with tile.TileContext(nc) as tc, ExitStack() as ctx:
        cpool = ctx.enter_context(tc.tile_pool(name="const", bufs=1))
        xpool = ctx.enter_context(tc.tile_pool(name="xtiles", bufs=2))
        work = ctx.enter_context(tc.tile_pool(name="work", bufs=2))

        ones = cpool.tile([1, 128], BF16, name="ones")
        nc.vector.memset(ones[:, :], 1.0)
        # broadcast s across partitions: psum = ones.T @ s chunk, then
        # ScalarE casts PSUM f32 -> SBUF bf16. s loaded via small tile.
        s_b = cpool.tile([128, TOTF], BF16, name="s_b")
        pspool = ctx.enter_context(tc.psum_pool(name="ps", bufs=4))
        for ld in range(TOTF // 4096):
            sld = xpool.tile([1, 4096], BF16, tag="sld", name=f"sld{ld}",
                             bufs=2)
            nc.sync.dma_start(out=sld[0:1, :],
                              in_=s_in[0:1, ld * 4096:(ld + 1) * 4096])
            for qq in range(8):
                q = ld * 8 + qq
                ps = pspool.tile([128, 512], F32, tag="ps", name=f"ps{q}",
                                 bufs=4)
                nc.tensor.matmul(out=ps[:, :], lhsT=ones[0:1, 0:128],
                                 rhs=sld[0:1, qq * 512:(qq + 1) * 512],
                                 start=True, stop=True)
                nc.scalar.copy(s_b[:, q * 512:(q + 1) * 512], ps[:, :])

        for r in range(reps):
          for t in range(NT):
            xs = xpool.tile([128, 2 * SRC], BF16, tag="xs", name=f"xs{r}_{t}",
                            bufs=2)
            nc.sync.dma_start(out=xs[:, :], in_=xs_in[t * 128:(t + 1) * 128, :])
            for pb in range(2):          # 0 = odd d, 1 = even d
                woff0 = SRC if pb == 0 else 2
                # DVE pieces: [0, 32) blocks, then [32, 32+DK) blocks
                pieces = [(0, ND // NCHUNK)]
                if DK > 0:
                    pieces.append((ND // NCHUNK, DK))
                for c, (k0, nk) in enumerate(pieces):
                    fsz = nk * NI
                    cbase = pb * HALF + k0 * NI
                    win = _wap(xs, woff0 + 2 * k0, [[2, nk], [1, NI]])
                    bca = _wap(xs, 0, [[0, nk], [1, NI]])
                    t1 = work.tile([128, CF], BF16, tag="t1",
                                   name=f"t{r}_{t}_{pb}_{c}", bufs=2)
                    nc.vector.tensor_mul(t1[:, 0:fsz], win, bca)
                    o = work.tile([128, CF], BF16, tag="o",
                                  name=f"o{r}_{t}_{pb}_{c}", bufs=2)
                    nc.vector.tensor_mul(o[:, 0:fsz], t1[:, 0:fsz],
                                         s_b[:, cbase:cbase + fsz])
                    nc.sync.dma_start(
                        out=out_t[t * 128:(t + 1) * 128, cbase:cbase + fsz],
                        in_=o[:, 0:fsz])
                if GK > 0:   # gpsimd tail slice, decoupled pipeline
                    k0 = ND - GK
                    fsz = GK * NI
                    cbase = pb * HALF + k0 * NI
                    win = _wap(xs, woff0 + 2 * k0, [[2, GK], [1, NI]])
                    bca = _wap(xs, 0, [[0, GK], [1, NI]])
                    t1g = work.tile([128, fsz], BF16, tag="t1g",
                                    name=f"tg{r}_{t}_{pb}", bufs=3)
                    nc.vector.tensor_mul(t1g[:, :], win, bca)
                    og = work.tile([128, fsz], BF16, tag="og",
                                   name=f"og{r}_{t}_{pb}", bufs=2)
                    nc.gpsimd.tensor_tensor(
                        og[:, :], t1g[:, :], s_b[:, cbase:cbase + fsz],
                        op=mybir.AluOpType.mult)
                    nc.sync.dma_start(
                        out=out_t[t * 128:(t + 1) * 128, cbase:cbase + fsz],
                        in_=og[:, :])

    nc.compile()
    return nc


class _Runner:
    def __init__(self, nc, n_cores=NCORES):
        install_neuronx_cc_hook()
        self.nc = nc
        self.n_cores = n_cores
        partition_name = (
            nc.partition_id_tensor.name if nc.partition_id_tensor else None
        )
        in_names, out_names, out_avals, zero_outs = [], [], [], []
        for alloc in nc.m.functions[0].allocations:
            if not isinstance(alloc, mybir.MemoryLocationSet):
                continue
            name = alloc.memorylocations[0].name
            if alloc.kind == "ExternalInput":
                if name != partition_name:
                    in_names.append(name)
            elif alloc.kind == "ExternalOutput":
                shape = tuple(alloc.tensor_shape)
                dtype = mybir.dt.np(alloc.dtype)
                out_avals.append(jax.core.ShapedArray(shape, dtype))
                zero_outs.append(np.zeros(shape, dtype))
                out_names.append(name)
        self.n_params = len(in_names)
        self.param_names = list(in_names)
        self.out_names = out_names
        self.out_avals = out_avals
        self.zero_outs = zero_outs
        all_in = in_names + out_names
        if partition_name is not None:
            all_in.append(partition_name)

        def _body(*args):
            operands = list(args)
            if partition_name is not None:
                operands.append(partition_id_tensor())
            return tuple(_bass_exec_p.bind(
                *operands,
                out_avals=tuple(out_avals),
                in_names=tuple(all_in),
                out_names=tuple(out_names),
                lowering_input_output_aliases=(),
                sim_require_finite=False,
                sim_require_nnan=False,
                nc=nc,
            ))

        devices = jax.devices()[:n_cores]
        mesh = Mesh(np.asarray(devices), ("core",))
        n_outs = len(out_names)
        in_specs = (PartitionSpec("core"),) * (self.n_params + n_outs)
        out_specs = (PartitionSpec("core"),) * n_outs
        self.fn = jax.jit(
            shard_map(_body, mesh=mesh, in_specs=in_specs,
                      out_specs=out_specs, check_rep=False),
            keep_unused=True,
        )

    def run_concat(self, concat_in):
        concat_zeros = [
            np.zeros((self.n_cores * z.shape[0], *z.shape[1:]), z.dtype)
            for z in self.zero_outs
        ]
        outs = self.fn(*concat_in, *concat_zeros)
        return [np.asarray(o) for o in outs]


_CACHE = {}


def _get_runner(reps=1, n_off=0):
    key = (reps, n_off)
    if key not in _CACHE:
        _CACHE[key] = _Runner(_build_nc(reps, n_off))
    return _CACHE[key]


def _host_prep(x, latent_emb):
    x = np.asarray(x, np.float32)
    L = np.asarray(latent_emb, np.float32)
    s = (L @ L.T).astype(np.float32)

    xb = x.astype(BF16NP)
    m = np.arange(SRC)
    xsrc = np.empty((B, 2 * SRC), BF16NP)
    xsrc[:, 0:SRC] = xb[:, m % NCOL]          # even-d source
    xsrc[:, SRC:2 * SRC] = xb[:, (m + 1) % NCOL]  # odd-d source

    # s_flat[(pb, k, i)]: pb=0: d=2k+1; pb=1: d=2k+2
    s_flat = np.zeros(TOTF, np.float32)
    for pb in range(2):
        for k in range(ND):
            d = 2 * k + 1 + pb
            i = np.arange(NI)
            j = (i + d) % NCOL
            v = s[i, j]
            if d == 128:
                v = np.where(i < 128, v, 0.0)
            s_flat[pb * HALF + k * NI:(pb * HALF + (k + 1) * NI)] = v
    s_row = s_flat.astype(BF16NP)[None, :]    # [1, TOTF]

    xs_cores = [np.ascontiguousarray(xsrc[c * BPC:(c + 1) * BPC])
                for c in range(NCORES)]
    return xs_cores, s_row


_IDX = None


def _pair_index():
    global _IDX
    if _IDX is None:
        iu, ju = np.triu_indices(NCOL, k=1)
        delta = ju - iu
        d = np.where(delta <= 128, delta, NCOL - delta)
        pos = np.where(delta <= 128, iu, ju)
        pb = np.where(d % 2 == 1, 0, 1)
        k = np.where(d % 2 == 1, (d - 1) // 2, d // 2 - 1)
        _IDX = (pb * HALF + k * NI + pos).astype(np.int64)
    return _IDX


def kernel(x, latent_emb):
    xs_cores, s_row = _host_prep(x, latent_emb)
    runner = _get_runner()
    concat_in = []
    for name in runner.param_names:
        if name == "xs":
            concat_in.append(np.concatenate(xs_cores, axis=0))
        elif name == "srow":
            concat_in.append(np.concatenate([s_row] * NCORES, axis=0))
        else:
            raise KeyError(name)
    outs = runner.run_concat(concat_in)
    dev = outs[runner.out_names.index("out")]     # [4096, TOTF] bf16
    return dev[:, _pair_index()].astype(np.float32)
